# revision 3
# baseline (speedup 1.0000x reference)
"""EvolveGCN Trainium2 kernel (8 NeuronCores, SPMD).

Structure
---------
The model is two GRCU layers scanned over T=8 timesteps. The only recurrent
state is the small GCN weight matrix W per layer; node embeddings at each
step are inputs (X[t] for layer 1, h1[t] for layer 2). Therefore:

  * The layer-1 weight trajectory Wn1[t] (and Y1[t] = X[t] @ Wn1[t]) is a
    pure function of the inputs — computed on host with jax-CPU, mirroring
    the reference ops so the top-k selections match the oracle bit-for-bit.
  * Launch 1 (the dominant compute, ~99% of FLOPs): per-core t computes
    h1[t] = relu(A[t] @ Y1[t]) — a [4096x4096] @ [4096x256] GEMM.
    Timestep-sharded: core t owns step t; A[t] is read from HBM exactly once.
  * The layer-2 trajectory (top-k over h1[t], GRU updates) runs on host from
    the returned h1, producing Y2 = h1[7] @ Wn2[7].
  * Launch 2: row-sharded final conv out = relu(A[7] @ Y2); core c computes
    512 rows.

Precision: the reference's top-k selections sit on score gaps as small as
~7e-6 (relative), so the GEMM must be fp32-grade. The PE's native fp32 path
runs at 4 cycles/row at reduced clock; instead each fp32 operand is split
into an fp16 hi + 2^11-scaled fp16 lo part and the product reconstructed
with three fp16 passes (T1 = hi@hi, T2 = hi@lo + lo@hi, result =
T1 + 2^-11*T2), giving ~3e-7 max relative error (slightly better than
a plain fp32 matmul) at fp16 throughput.

Matmul orientation: lhsT (stationary) = Y (node-major, natural layout),
rhs (moving) = A[t].T tiles [128 x 512] — wide moving operand, few weight
reloads. Output is h1[t].T, un-transposed on host for free.
"""
import os
import sys
import subprocess
import tempfile

import numpy as np

for _p in ("/opt/trn_rl_repo", "/root/.axon_site/_ro/trn_rl_repo"):
    if os.path.isdir(_p) and _p not in sys.path:
        sys.path.insert(0, _p)

import concourse.bass as bass  # noqa: E402
from concourse import bacc  # noqa: E402
import concourse.mybir as mybir  # noqa: E402
import concourse.tile as tile  # noqa: E402
from concourse.bass_utils import run_bass_kernel_spmd  # noqa: E402

T, N, D0, D1, D2 = 8, 4096, 256, 256, 128
NCORES = 8
ROWS = N // NCORES  # launch-2 rows per core
SCALE = float(2.0 ** -11)

# ---------------------------------------------------------------- host helper
# jax must run on CPU for the trajectory math (to mirror the oracle), but the
# parent process needs the neuron/axon platform for the bass launches, so the
# CPU part runs in a subprocess.
_HELPER_SRC = r'''
import sys
import numpy as np
import jax
jax.config.update("jax_platforms", "cpu")
import jax.numpy as jnp


def traj(Xs, masks, p, k):
    sc = p["scorer"]
    nrm = jnp.linalg.norm(sc)

    def step(W, inp):
        emb, m = inp
        scores = (emb @ sc)[:, 0] / nrm + m[:, 0]
        vals, idx = jax.lax.top_k(scores, k)
        H = (emb[idx] * jnp.tanh(vals)[:, None]).T
        upd = jax.nn.sigmoid(p["Wu"] @ H + p["Uu"] @ W + p["bu"])
        rst = jax.nn.sigmoid(p["Wr"] @ H + p["Ur"] @ W + p["br"])
        hcap = jnp.tanh(p["Wh"] @ H + p["Uh"] @ (rst * W) + p["bh"])
        Wn = (1.0 - upd) * W + upd * hcap
        y = emb @ Wn
        return Wn, y

    _, Ys = jax.lax.scan(step, p["Wg"], (Xs, masks))
    return Ys


mode, inp, outp = sys.argv[1:4]
z = np.load(inp)
pfx = "p1_" if mode == "w1" else "p2_"
p = {k[len(pfx):]: jnp.asarray(z[k]) for k in z.files if k.startswith(pfx)}
emb = jnp.asarray(z["X"] if mode == "w1" else z["h1"])
masks = jnp.asarray(z["mask"])
k = 256 if mode == "w1" else 128
Ys = traj(emb, masks, p, k)
if mode == "w1":
    np.savez(outp, Y=np.asarray(Ys))
else:
    np.savez(outp, Y=np.asarray(Ys[-1]))
'''


def _run_helper(mode, payload):
    with tempfile.TemporaryDirectory() as td:
        inp = os.path.join(td, "in.npz")
        outp = os.path.join(td, "out.npz")
        script = os.path.join(td, "helper.py")
        np.savez(inp, **payload)
        with open(script, "w") as f:
            f.write(_HELPER_SRC)
        env = dict(os.environ)
        subprocess.run([sys.executable, script, mode, inp, outp],
                       check=True, env=env, capture_output=True)
        with np.load(outp) as z:
            return np.asarray(z["Y"])


def _split16(x):
    """fp32 -> (hi, lo) fp16 with lo pre-scaled by 2^11 (x ~= hi + 2^-11 lo)."""
    h = x.astype(np.float16)
    l = ((x - h.astype(np.float32)) * 2048.0).astype(np.float16)
    return h, l


# ---------------------------------------------------------------- device side
_NC_CACHE = {}


def _loop(tc, r):
    """Context that repeats the body r times on-device (r>1: timing only)."""
    if r == 1:
        import contextlib
        return contextlib.nullcontext()
    return tc.For_i(0, r, 1, hint_engines=(mybir.EngineType.PE, mybir.EngineType.SP,
                                           mybir.EngineType.Activation,
                                           mybir.EngineType.DVE))


def _build_launch1(loop_r=1):
    """Per core: h1T [D1, N] = relu(Y.T @ AT) with split-fp16 operands.

    at: [2, N, N] fp16  (A[t].T hi/lo)   y: [2, N, D1] fp16 (Y1[t] hi/lo)
    """
    key = ("l1", loop_r)
    if key in _NC_CACHE:
        return _NC_CACHE[key]
    KT = N // 128
    NCH = 512
    NNCH = N // NCH
    MT = D1 // 128

    nc = bacc.Bacc(None, target_bir_lowering=False)
    at_d = nc.dram_tensor("at", [2, N, N], mybir.dt.float16, kind="ExternalInput")
    y_d = nc.dram_tensor("y", [2, N, D1], mybir.dt.float16, kind="ExternalInput")
    h1t_d = nc.dram_tensor("h1t", [D1, N], mybir.dt.float32, kind="ExternalOutput")

    with tile.TileContext(nc) as tc:
        with (
            tc.tile_pool(name="atp", bufs=12) as atp,
            tc.tile_pool(name="yp", bufs=1) as yp,
            tc.tile_pool(name="op", bufs=4) as op,
            tc.tile_pool(name="ps", bufs=2, space="PSUM") as ps,
        ):
            with _loop(tc, loop_r):
                yh = yp.tile([128, KT, D1], mybir.dt.float16, tag="yh")
                yl = yp.tile([128, KT, D1], mybir.dt.float16, tag="yl")
                for kt in range(KT):
                    nc.sync.dma_start(yh[:, kt, :], y_d[0, kt * 128:(kt + 1) * 128, :])
                    nc.sync.dma_start(yl[:, kt, :], y_d[1, kt * 128:(kt + 1) * 128, :])
                for nch in range(NNCH):
                    nsl = slice(nch * NCH, (nch + 1) * NCH)
                    ps1 = [ps.tile([128, NCH], mybir.dt.float32, tag=f"t1_{m}", name=f"t1_{m}")
                           for m in range(MT)]
                    ps2 = [ps.tile([128, NCH], mybir.dt.float32, tag=f"t2_{m}", name=f"t2_{m}")
                           for m in range(MT)]
                    for kt in range(KT):
                        ah = atp.tile([128, NCH], mybir.dt.float16, tag="ah")
                        nc.sync.dma_start(ah, at_d[0, kt * 128:(kt + 1) * 128, nsl])
                        al = atp.tile([128, NCH], mybir.dt.float16, tag="al")
                        nc.sync.dma_start(al, at_d[1, kt * 128:(kt + 1) * 128, nsl])
                        st, sp = (kt == 0), (kt == KT - 1)
                        for m in range(MT):
                            msl = slice(m * 128, (m + 1) * 128)
                            nc.tensor.matmul(ps1[m], yh[:, kt, msl], ah, start=st, stop=sp)
                            nc.tensor.matmul(ps2[m], yh[:, kt, msl], al, start=st, stop=False)
                            nc.tensor.matmul(ps2[m], yl[:, kt, msl], ah, start=False, stop=sp)
                    for m in range(MT):
                        t2s = op.tile([128, NCH], mybir.dt.float32, tag="t2s")
                        nc.scalar.activation(t2s, ps2[m], mybir.ActivationFunctionType.Copy,
                                             scale=SCALE)
                        s = op.tile([128, NCH], mybir.dt.float32, tag="sum")
                        nc.vector.tensor_add(s, ps1[m], t2s)
                        ot = op.tile([128, NCH], mybir.dt.float32, tag="out")
                        nc.scalar.activation(ot, s, mybir.ActivationFunctionType.Relu)
                        nc.sync.dma_start(h1t_d[m * 128:(m + 1) * 128, nsl], ot)
    nc.finalize()
    _NC_CACHE[key] = nc
    return nc


def _build_launch2(loop_r=1):
    """Per core c: outT [D2, ROWS] = relu(Y2.T @ AT7[:, c's rows]) split-fp16.

    at7: [2, N, ROWS] fp16   y2: [2, N, D2] fp16
    """
    key = ("l2", loop_r)
    if key in _NC_CACHE:
        return _NC_CACHE[key]
    KT = N // 128

    nc = bacc.Bacc(None, target_bir_lowering=False)
    at_d = nc.dram_tensor("at7", [2, N, ROWS], mybir.dt.float16, kind="ExternalInput")
    y_d = nc.dram_tensor("y2", [2, N, D2], mybir.dt.float16, kind="ExternalInput")
    out_d = nc.dram_tensor("outT", [D2, ROWS], mybir.dt.float32, kind="ExternalOutput")

    with tile.TileContext(nc) as tc:
        with (
            tc.tile_pool(name="atp", bufs=12) as atp,
            tc.tile_pool(name="yp", bufs=1) as yp,
            tc.tile_pool(name="op", bufs=2) as op,
            tc.tile_pool(name="ps", bufs=1, space="PSUM") as ps,
        ):
            with _loop(tc, loop_r):
                yh = yp.tile([128, KT, D2], mybir.dt.float16, tag="yh")
                yl = yp.tile([128, KT, D2], mybir.dt.float16, tag="yl")
                for kt in range(KT):
                    nc.sync.dma_start(yh[:, kt, :], y_d[0, kt * 128:(kt + 1) * 128, :])
                    nc.sync.dma_start(yl[:, kt, :], y_d[1, kt * 128:(kt + 1) * 128, :])
                ps1 = ps.tile([128, ROWS], mybir.dt.float32, tag="t1")
                ps2 = ps.tile([128, ROWS], mybir.dt.float32, tag="t2")
                for kt in range(KT):
                    ah = atp.tile([128, ROWS], mybir.dt.float16, tag="ah")
                    nc.sync.dma_start(ah, at_d[0, kt * 128:(kt + 1) * 128, :])
                    al = atp.tile([128, ROWS], mybir.dt.float16, tag="al")
                    nc.sync.dma_start(al, at_d[1, kt * 128:(kt + 1) * 128, :])
                    st, sp = (kt == 0), (kt == KT - 1)
                    nc.tensor.matmul(ps1, yh[:, kt, :], ah, start=st, stop=sp)
                    nc.tensor.matmul(ps2, yh[:, kt, :], al, start=st, stop=False)
                    nc.tensor.matmul(ps2, yl[:, kt, :], ah, start=False, stop=sp)
                t2s = op.tile([128, ROWS], mybir.dt.float32, tag="t2s")
                nc.scalar.activation(t2s, ps2, mybir.ActivationFunctionType.Copy, scale=SCALE)
                s = op.tile([128, ROWS], mybir.dt.float32, tag="sum")
                nc.vector.tensor_add(s, ps1, t2s)
                ot = op.tile([128, ROWS], mybir.dt.float32, tag="out")
                nc.scalar.activation(ot, s, mybir.ActivationFunctionType.Relu)
                nc.sync.dma_start(out_d[:, :], ot)
    nc.finalize()
    _NC_CACHE[key] = nc
    return nc


# ------------------------------------------------------------------- kernel()
def kernel(A, X, mask, params1, params2):
    A = np.asarray(A, dtype=np.float32)
    X = np.asarray(X, dtype=np.float32)
    mask = np.asarray(mask, dtype=np.float32)
    p1 = {k: np.asarray(v, dtype=np.float32) for k, v in params1.items()}
    p2 = {k: np.asarray(v, dtype=np.float32) for k, v in params2.items()}

    # 1) layer-1 weight trajectory + Y1[t] = X[t] @ Wn1[t]  (host, jax-CPU)
    Y1 = _run_helper("w1", {"X": X, "mask": mask,
                            **{f"p1_{k}": v for k, v in p1.items()}})

    # 2) launch 1: h1[t] = relu(A[t] @ Y1[t]), timestep t -> core t
    AT = np.ascontiguousarray(A.transpose(0, 2, 1))
    ATh, ATl = _split16(AT)
    in_maps = []
    for t in range(T):
        yh, yl = _split16(Y1[t])
        in_maps.append({"at": np.stack([ATh[t], ATl[t]]),
                        "y": np.stack([yh, yl])})
    nc1 = _build_launch1()
    res1 = run_bass_kernel_spmd(nc1, in_maps, core_ids=list(range(NCORES)))
    h1 = np.stack([np.ascontiguousarray(res1.results[t]["h1t"].T) for t in range(T)])

    # 3) layer-2 trajectory from h1 -> Y2 = h1[7] @ Wn2[7]  (host, jax-CPU)
    Y2 = _run_helper("w2", {"h1": h1, "mask": mask,
                            **{f"p2_{k}": v for k, v in p2.items()}})

    # 4) launch 2: out = relu(A[7] @ Y2), rows sharded across cores
    y2h, y2l = _split16(Y2)
    y2s = np.stack([y2h, y2l])
    in_maps2 = []
    for c in range(NCORES):
        csl = slice(c * ROWS, (c + 1) * ROWS)
        in_maps2.append({"at7": np.ascontiguousarray(
                             np.stack([ATh[7][:, csl], ATl[7][:, csl]])),
                         "y2": y2s})
    nc2 = _build_launch2()
    res2 = run_bass_kernel_spmd(nc2, in_maps2, core_ids=list(range(NCORES)))
    out = np.concatenate([np.ascontiguousarray(res2.results[c]["outT"].T)
                          for c in range(NCORES)], axis=0)
    return out.astype(np.float32)


# revision 5
# speedup vs baseline: 1.3075x; 1.3075x over previous
"""EvolveGCN Trainium2 kernel (8 NeuronCores, SPMD).

Structure
---------
The model is two GRCU layers scanned over T=8 timesteps. The only recurrent
state is the small GCN weight matrix W per layer; node embeddings at each
step are inputs (X[t] for layer 1, h1[t] for layer 2). Therefore:

  * The layer-1 weight trajectory Wn1[t] (and Y1[t] = X[t] @ Wn1[t]) is a
    pure function of the inputs — computed on host with jax-CPU, mirroring
    the reference ops so the top-k selections match the oracle bit-for-bit.
  * Launch 1 (the dominant compute, ~99% of FLOPs): per-core t computes
    h1[t] = relu(A[t] @ Y1[t]) — a [4096x4096] @ [4096x256] GEMM.
    Timestep-sharded: core t owns step t; A[t] is read from HBM exactly once.
  * The layer-2 trajectory (top-k over h1[t], GRU updates) runs on host from
    the returned h1, producing Y2 = h1[7] @ Wn2[7].
  * Launch 2: row-sharded final conv out = relu(A[7] @ Y2); core c computes
    512 rows.

Precision: the reference's top-k selections sit on score gaps as small as
~7e-6 (relative), so the GEMM must be fp32-grade. The PE's native fp32 path
runs at 4 cycles/row at reduced clock; instead each fp32 operand is split
into an fp16 hi + 2^11-scaled fp16 lo part and the product reconstructed
with three fp16 passes (T1 = hi@hi, T2 = hi@lo + lo@hi, result =
T1 + 2^-11*T2), giving ~3e-7 max relative error (slightly better than
a plain fp32 matmul) at fp16 throughput.

Matmul orientation: lhsT (stationary) = Y (node-major, natural layout),
rhs (moving) = A[t].T tiles [128 x 512] — wide moving operand, few weight
reloads. Output is h1[t].T, un-transposed on host for free.
"""
import os
import sys
import subprocess
import tempfile

import numpy as np

for _p in ("/opt/trn_rl_repo", "/root/.axon_site/_ro/trn_rl_repo"):
    if os.path.isdir(_p) and _p not in sys.path:
        sys.path.insert(0, _p)

import concourse.bass as bass  # noqa: E402
from concourse import bacc  # noqa: E402
import concourse.mybir as mybir  # noqa: E402
import concourse.tile as tile  # noqa: E402
from concourse.bass_utils import run_bass_kernel_spmd  # noqa: E402

T, N, D0, D1, D2 = 8, 4096, 256, 256, 128
NCORES = 8
ROWS = N // NCORES  # launch-2 rows per core
SCALE = float(2.0 ** -11)

# ---------------------------------------------------------------- host helper
# jax must run on CPU for the trajectory math (to mirror the oracle), but the
# parent process needs the neuron/axon platform for the bass launches, so the
# CPU part runs in a subprocess.
_HELPER_SRC = r'''
import sys
import numpy as np
import jax
jax.config.update("jax_platforms", "cpu")
import jax.numpy as jnp


def traj(Xs, masks, p, k):
    sc = p["scorer"]
    nrm = jnp.linalg.norm(sc)

    def step(W, inp):
        emb, m = inp
        scores = (emb @ sc)[:, 0] / nrm + m[:, 0]
        vals, idx = jax.lax.top_k(scores, k)
        H = (emb[idx] * jnp.tanh(vals)[:, None]).T
        upd = jax.nn.sigmoid(p["Wu"] @ H + p["Uu"] @ W + p["bu"])
        rst = jax.nn.sigmoid(p["Wr"] @ H + p["Ur"] @ W + p["br"])
        hcap = jnp.tanh(p["Wh"] @ H + p["Uh"] @ (rst * W) + p["bh"])
        Wn = (1.0 - upd) * W + upd * hcap
        y = emb @ Wn
        return Wn, y

    _, Ys = jax.lax.scan(step, p["Wg"], (Xs, masks))
    return Ys


mode, inp, outp = sys.argv[1:4]
z = np.load(inp)
pfx = "p1_" if mode == "w1" else "p2_"
p = {k[len(pfx):]: jnp.asarray(z[k]) for k in z.files if k.startswith(pfx)}
emb = jnp.asarray(z["X"] if mode == "w1" else z["h1"])
masks = jnp.asarray(z["mask"])
k = 256 if mode == "w1" else 128
Ys = traj(emb, masks, p, k)
if mode == "w1":
    np.savez(outp, Y=np.asarray(Ys))
else:
    np.savez(outp, Y=np.asarray(Ys[-1]))
'''


def _run_helper(mode, payload):
    with tempfile.TemporaryDirectory() as td:
        inp = os.path.join(td, "in.npz")
        outp = os.path.join(td, "out.npz")
        script = os.path.join(td, "helper.py")
        np.savez(inp, **payload)
        with open(script, "w") as f:
            f.write(_HELPER_SRC)
        env = dict(os.environ)
        subprocess.run([sys.executable, script, mode, inp, outp],
                       check=True, env=env, capture_output=True)
        with np.load(outp) as z:
            return np.asarray(z["Y"])


def _split16(x):
    """fp32 -> (hi, lo) fp16 with lo pre-scaled by 2^11 (x ~= hi + 2^-11 lo)."""
    h = x.astype(np.float16)
    l = ((x - h.astype(np.float32)) * 2048.0).astype(np.float16)
    return h, l


# ---------------------------------------------------------------- device side
_NC_CACHE = {}


def _loop(tc, r):
    """Context that repeats the body r times on-device (r>1: timing only)."""
    if r == 1:
        import contextlib
        return contextlib.nullcontext()
    return tc.For_i(0, r, 1, hint_engines=(mybir.EngineType.PE, mybir.EngineType.SP,
                                           mybir.EngineType.Activation,
                                           mybir.EngineType.DVE))


def _build_launch1(loop_r=1):
    """Per core: h1T [D1, N] = relu(Y.T @ AT) with split-fp16 operands.

    at: [2, N, N] fp16  (A[t].T hi/lo)   y: [2, N, D1] fp16 (Y1[t] hi/lo)
    """
    key = ("l1", loop_r)
    if key in _NC_CACHE:
        return _NC_CACHE[key]
    KT = N // 128
    NCH = 512
    NNCH = N // NCH
    MT = D1 // 128

    nc = bacc.Bacc(None, target_bir_lowering=False)
    at_d = nc.dram_tensor("at", [2, N, N], mybir.dt.float16, kind="ExternalInput")
    y_d = nc.dram_tensor("y", [2, N, D1], mybir.dt.float16, kind="ExternalInput")
    h1t_d = nc.dram_tensor("h1t", [D1, N], mybir.dt.float32, kind="ExternalOutput")

    with tile.TileContext(nc) as tc:
        with (
            tc.tile_pool(name="atp", bufs=12) as atp,
            tc.tile_pool(name="yp", bufs=1) as yp,
            tc.tile_pool(name="op", bufs=4) as op,
            tc.tile_pool(name="ps", bufs=2, space="PSUM") as ps,
        ):
            with _loop(tc, loop_r):
                # Y preload: 2 big DMAs ("(kt p) d -> p kt d" rearranged view)
                yh = yp.tile([128, KT, D1], mybir.dt.float16, tag="yh")
                yl = yp.tile([128, KT, D1], mybir.dt.float16, tag="yl")
                yv = y_d.rearrange("s (kt p) d -> s p kt d", p=128)
                nc.sync.dma_start(yh, yv[0])
                nc.sync.dma_start(yl, yv[1])
                # A view: hi/lo interleaved per (kt, nch) tile
                av = at_d.rearrange("s (kt p) n -> p s kt n", p=128)
                for nch in range(NNCH):
                    nsl = slice(nch * NCH, (nch + 1) * NCH)
                    ps1 = [ps.tile([128, NCH], mybir.dt.float32, tag=f"t1_{m}", name=f"t1_{m}")
                           for m in range(MT)]
                    ps2 = [ps.tile([128, NCH], mybir.dt.float32, tag=f"t2_{m}", name=f"t2_{m}")
                           for m in range(MT)]
                    for kt in range(KT):
                        a_t = atp.tile([128, 2, NCH], mybir.dt.float16, tag="a_t")
                        eng = nc.sync if kt % 2 == 0 else nc.scalar
                        eng.dma_start(a_t, av[:, :, kt, nsl])
                        ah, al = a_t[:, 0, :], a_t[:, 1, :]
                        st, sp = (kt == 0), (kt == KT - 1)
                        for m in range(MT):
                            msl = slice(m * 128, (m + 1) * 128)
                            nc.tensor.matmul(ps1[m], yh[:, kt, msl], ah, start=st, stop=sp)
                            nc.tensor.matmul(ps2[m], yh[:, kt, msl], al, start=st, stop=False)
                            nc.tensor.matmul(ps2[m], yl[:, kt, msl], ah, start=False, stop=sp)
                    for m in range(MT):
                        t2s = op.tile([128, NCH], mybir.dt.float32, tag="t2s")
                        nc.scalar.activation(t2s, ps2[m], mybir.ActivationFunctionType.Copy,
                                             scale=SCALE)
                        s = op.tile([128, NCH], mybir.dt.float32, tag="sum")
                        nc.vector.tensor_add(s, ps1[m], t2s)
                        ot = op.tile([128, NCH], mybir.dt.float32, tag="out")
                        nc.scalar.activation(ot, s, mybir.ActivationFunctionType.Relu)
                        nc.sync.dma_start(h1t_d[m * 128:(m + 1) * 128, nsl], ot)
    nc.finalize()
    _NC_CACHE[key] = nc
    return nc


def _build_launch2(loop_r=1):
    """Per core c: outT [D2, ROWS] = relu(Y2.T @ AT7[:, c's rows]) split-fp16.

    at7: [2, N, ROWS] fp16   y2: [2, N, D2] fp16
    """
    key = ("l2", loop_r)
    if key in _NC_CACHE:
        return _NC_CACHE[key]
    KT = N // 128

    nc = bacc.Bacc(None, target_bir_lowering=False)
    at_d = nc.dram_tensor("at7", [2, N, ROWS], mybir.dt.float16, kind="ExternalInput")
    y_d = nc.dram_tensor("y2", [2, N, D2], mybir.dt.float16, kind="ExternalInput")
    out_d = nc.dram_tensor("outT", [D2, ROWS], mybir.dt.float32, kind="ExternalOutput")

    with tile.TileContext(nc) as tc:
        with (
            tc.tile_pool(name="atp", bufs=12) as atp,
            tc.tile_pool(name="yp", bufs=1) as yp,
            tc.tile_pool(name="op", bufs=2) as op,
            tc.tile_pool(name="ps", bufs=1, space="PSUM") as ps,
        ):
            with _loop(tc, loop_r):
                yh = yp.tile([128, KT, D2], mybir.dt.float16, tag="yh")
                yl = yp.tile([128, KT, D2], mybir.dt.float16, tag="yl")
                yv = y_d.rearrange("s (kt p) d -> s p kt d", p=128)
                nc.sync.dma_start(yh, yv[0])
                nc.sync.dma_start(yl, yv[1])
                av = at_d.rearrange("s (kt p) n -> p s kt n", p=128)
                ps1 = ps.tile([128, ROWS], mybir.dt.float32, tag="t1")
                ps2 = ps.tile([128, ROWS], mybir.dt.float32, tag="t2")
                for kt in range(KT):
                    a_t = atp.tile([128, 2, ROWS], mybir.dt.float16, tag="a_t")
                    eng = nc.sync if kt % 2 == 0 else nc.scalar
                    eng.dma_start(a_t, av[:, :, kt, :])
                    ah, al = a_t[:, 0, :], a_t[:, 1, :]
                    st, sp = (kt == 0), (kt == KT - 1)
                    nc.tensor.matmul(ps1, yh[:, kt, :], ah, start=st, stop=sp)
                    nc.tensor.matmul(ps2, yh[:, kt, :], al, start=st, stop=False)
                    nc.tensor.matmul(ps2, yl[:, kt, :], ah, start=False, stop=sp)
                t2s = op.tile([128, ROWS], mybir.dt.float32, tag="t2s")
                nc.scalar.activation(t2s, ps2, mybir.ActivationFunctionType.Copy, scale=SCALE)
                s = op.tile([128, ROWS], mybir.dt.float32, tag="sum")
                nc.vector.tensor_add(s, ps1, t2s)
                ot = op.tile([128, ROWS], mybir.dt.float32, tag="out")
                nc.scalar.activation(ot, s, mybir.ActivationFunctionType.Relu)
                nc.sync.dma_start(out_d[:, :], ot)
    nc.finalize()
    _NC_CACHE[key] = nc
    return nc


# ------------------------------------------------------------------- kernel()
def kernel(A, X, mask, params1, params2):
    A = np.asarray(A, dtype=np.float32)
    X = np.asarray(X, dtype=np.float32)
    mask = np.asarray(mask, dtype=np.float32)
    p1 = {k: np.asarray(v, dtype=np.float32) for k, v in params1.items()}
    p2 = {k: np.asarray(v, dtype=np.float32) for k, v in params2.items()}

    # 1) layer-1 weight trajectory + Y1[t] = X[t] @ Wn1[t]  (host, jax-CPU)
    Y1 = _run_helper("w1", {"X": X, "mask": mask,
                            **{f"p1_{k}": v for k, v in p1.items()}})

    # 2) launch 1: h1[t] = relu(A[t] @ Y1[t]), timestep t -> core t
    AT = np.ascontiguousarray(A.transpose(0, 2, 1))
    ATh, ATl = _split16(AT)
    in_maps = []
    for t in range(T):
        yh, yl = _split16(Y1[t])
        in_maps.append({"at": np.stack([ATh[t], ATl[t]]),
                        "y": np.stack([yh, yl])})
    nc1 = _build_launch1()
    res1 = run_bass_kernel_spmd(nc1, in_maps, core_ids=list(range(NCORES)))
    h1 = np.stack([np.ascontiguousarray(res1.results[t]["h1t"].T) for t in range(T)])

    # 3) layer-2 trajectory from h1 -> Y2 = h1[7] @ Wn2[7]  (host, jax-CPU)
    Y2 = _run_helper("w2", {"h1": h1, "mask": mask,
                            **{f"p2_{k}": v for k, v in p2.items()}})

    # 4) launch 2: out = relu(A[7] @ Y2), rows sharded across cores
    y2h, y2l = _split16(Y2)
    y2s = np.stack([y2h, y2l])
    in_maps2 = []
    for c in range(NCORES):
        csl = slice(c * ROWS, (c + 1) * ROWS)
        in_maps2.append({"at7": np.ascontiguousarray(
                             np.stack([ATh[7][:, csl], ATl[7][:, csl]])),
                         "y2": y2s})
    nc2 = _build_launch2()
    res2 = run_bass_kernel_spmd(nc2, in_maps2, core_ids=list(range(NCORES)))
    out = np.concatenate([np.ascontiguousarray(res2.results[c]["outT"].T)
                          for c in range(NCORES)], axis=0)
    return out.astype(np.float32)
